# revision 2
# baseline (speedup 1.0000x reference)
"""BasisConv GNN message passing on 8 TRN2 NeuronCores.

Edge-parallel: edges sorted by dst, split equally (not node-aligned) across
8 cores; tiles are exactly 128 consecutive edges. Each (tile, run-of-equal-dst)
gets a unique output row per core; host merges rows (np.add.at) at the end.

Device per chunk (16 tiles): one batched indirect gather of x_j rows (f16,
from an AllGather'd replica), hat-basis + one-hot segment matrices on DVE,
PE pipeline (transpose -> feat@W -> *b -> one-hot segment-sum), one batched
indirect scatter of f16 rows to a compact per-core output.

Dispatch path: cached jax.jit(shard_map(bass_exec)) — compiled once per
(T, ROWS); per-call work is host->device upload of ~13MB of inputs, NEFF
execution, ~4MB readback.
"""

import sys
import time

for _p in ("/opt/trn_rl_repo", "/opt/pypackages"):
    if _p not in sys.path:
        sys.path.insert(0, _p)

import numpy as np

import concourse.bacc as bacc
import concourse.bass as bass
import concourse.mybir as mybir
import concourse.tile as tile
from concourse import bass2jax

N_NODES = 50000
F = 32          # feature dim (in == out)
NB = 4          # basis terms per dimension
K = NB * NB     # 16 mixture terms
P = 128         # edges per tile
SEG = 32        # max runs (output rows) per tile
CH = 16         # tiles per chunk (one gather/scatter DMA per chunk)
GRP = 4         # tiles per PE-transpose / PSUM column group
NG = CH // GRP
NCORES = 8
NSH = N_NODES // NCORES          # 6250 nodes per x_j shard
DX = 2.0 / (NB - 1)
DUMMY = 99.0                     # basis value is exactly 0 out there
ROWS_DEFAULT = 7168

f32 = mybir.dt.float32
f16 = mybir.dt.float16
i32 = mybir.dt.int32
u16 = mybir.dt.uint16
u8 = mybir.dt.uint8

LAST_DISPATCH_NS = None
_EXEC = {}
_SHD = {}


def _build_nc(T, ROWS, debug_dump=False):
    assert T % CH == 0
    nc = bacc.Bacc("TRN2", target_bir_lowering=False, debug=False,
                   enable_asserts=False, num_devices=NCORES)
    AO = mybir.AluOpType
    AF = mybir.ActivationFunctionType
    dbg = {}
    if debug_dump:
        dbg["xjf"] = nc.dram_tensor("dbg_xjf", [N_NODES, F], f16,
                                    kind="ExternalOutput")
        dbg["ident"] = nc.dram_tensor("dbg_ident", [P, P], f32,
                                      kind="ExternalOutput")
        dbg["cen"] = nc.dram_tensor("dbg_cen", [P, 2 * NB], f32,
                                    kind="ExternalOutput")
        dbg["io"] = nc.dram_tensor("dbg_io", [P, SEG], f32,
                                   kind="ExternalOutput")
        dbg["idx"] = nc.dram_tensor("dbg_idx", [P, CH], i32,
                                    kind="ExternalOutput")
        dbg["feat"] = nc.dram_tensor("dbg_feat", [P, CH * F], f16,
                                     kind="ExternalOutput")
        dbg["stage"] = nc.dram_tensor("dbg_stage", [P, NG * F], f16,
                                      kind="ExternalOutput")
        dbg["nid"] = nc.dram_tensor("dbg_nid", [P, NG], i32,
                                    kind="ExternalOutput")

    xjs_d = nc.dram_tensor("xjs", [NSH, F], f16, kind="ExternalInput")
    src_d = nc.dram_tensor("src_il", [P, T], u16, kind="ExternalInput")
    attr_d = nc.dram_tensor("attr_il", [P, T * 2], f16, kind="ExternalInput")
    seg_d = nc.dram_tensor("seg_il", [P, T], u8, kind="ExternalInput")
    nid_d = nc.dram_tensor("nid_il", [P, T // GRP], u16, kind="ExternalInput")
    wf_d = nc.dram_tensor("wflat", [F, K * F], f32, kind="ExternalInput")
    out_d = nc.dram_tensor("out", [ROWS, F], f16, kind="ExternalOutput")

    NCH = T // CH
    with tile.TileContext(nc) as tc:
        with (
            tc.tile_pool(name="dram", bufs=1, space="DRAM") as dpool,
            tc.tile_pool(name="const", bufs=1) as cpool,
            tc.tile_pool(name="io", bufs=2) as iopool,
            tc.tile_pool(name="work", bufs=2) as wpool,
            tc.tile_pool(name="zzp", bufs=6) as zzpool,
            tc.tile_pool(name="ftp", bufs=2, space="PSUM") as ftpool,
            tc.tile_pool(name="yp", bufs=4, space="PSUM") as ypool,
            tc.tile_pool(name="sp", bufs=2, space="PSUM") as spool,
        ):
            # x_j: shard -> replicated full table (f16) in local DRAM
            xj_bounce = dpool.tile([NSH, F], f16, tag="xjb")
            xj_full = dpool.tile([N_NODES, F], f16, tag="xjf")
            nc.gpsimd.dma_start(xj_bounce[:], xjs_d[:, :])
            nc.gpsimd.collective_compute(
                "AllGather", AO.bypass,
                replica_groups=[list(range(NCORES))],
                ins=[xj_bounce.opt()], outs=[xj_full.opt()])

            # constants: W replicated to 128 rows; ident/centers/iota on-device
            wf = cpool.tile([P, K * F], f32, tag="wf")
            for j in range(GRP):
                nc.sync.dma_start(wf[32 * j:32 * (j + 1), :], wf_d[:, :])

            ident_i = cpool.tile([P, P], i32, tag="identi")
            identf = cpool.tile([P, P], f32, tag="identf")
            ident = cpool.tile([P, P], f32, tag="ident")
            nc.gpsimd.iota(ident_i[:], [[1, P]], channel_multiplier=-1)
            nc.vector.tensor_copy(identf[:], ident_i[:])
            nc.vector.tensor_scalar(
                out=ident[:], in0=identf[:], scalar1=0.0, scalar2=None,
                op0=AO.is_equal)

            cen_i = cpool.tile([P, 2 * NB], i32, tag="ceni")
            cen = cpool.tile([P, 2 * NB], f32, tag="cen")
            nc.gpsimd.iota(cen_i[:], [[0, 2], [1, NB]], channel_multiplier=0)
            nc.vector.tensor_scalar(
                out=cen[:], in0=cen_i[:], scalar1=float(DX), scalar2=-1.0,
                op0=AO.mult, op1=AO.add)

            io_i = cpool.tile([P, SEG], i32, tag="ioi")
            io32 = cpool.tile([P, SEG], f32, tag="io")
            nc.gpsimd.iota(io_i[:], [[1, SEG]], channel_multiplier=0)
            nc.vector.tensor_copy(io32[:], io_i[:])

            if debug_dump:
                nc.sync.dma_start(dbg["xjf"][:, :], xj_full[:])
                nc.sync.dma_start(dbg["ident"][:, :], ident[:])
                nc.sync.dma_start(dbg["cen"][:, :], cen[:])
                nc.sync.dma_start(dbg["io"][:, :], io32[:])

            for c in range(NCH):
                ts = slice(c * CH, (c + 1) * CH)
                idx16 = iopool.tile([P, CH], u16, tag="idx16")
                attr16 = iopool.tile([P, CH * 2], f16, tag="attr16")
                seg8 = iopool.tile([P, CH], u8, tag="seg8")
                nid16 = iopool.tile([P, NG], u16, tag="nid16")
                nc.sync.dma_start(idx16[:], src_d[:, ts])
                nc.sync.dma_start(attr16[:], attr_d[:, c * CH * 2:(c + 1) * CH * 2])
                nc.sync.dma_start(seg8[:], seg_d[:, ts])
                nc.sync.dma_start(nid16[:], nid_d[:, c * NG:(c + 1) * NG])

                idx = iopool.tile([P, CH], i32, tag="idx")
                attrf = wpool.tile([P, CH * 2], f32, tag="attrf")
                segf = wpool.tile([P, CH], f32, tag="segf")
                nid = iopool.tile([P, NG], i32, tag="nid")
                nc.vector.tensor_copy(idx[:], idx16[:])
                nc.vector.tensor_copy(attrf[:], attr16[:])
                nc.vector.tensor_copy(segf[:], seg8[:])
                nc.vector.tensor_copy(nid[:], nid16[:])

                feat16 = wpool.tile([P, CH * F], f16, tag="feat16")
                for tl in range(CH):
                    nc.gpsimd.indirect_dma_start(
                        out=feat16[:, tl * F:(tl + 1) * F], out_offset=None,
                        in_=xj_full[:, :],
                        in_offset=bass.IndirectOffsetOnAxis(
                            ap=idx[:, tl:tl + 1], axis=0))
                feat = wpool.tile([P, CH * F], f32, tag="feat")
                nc.vector.tensor_copy(feat[:], feat16[:])

                # hat basis for the whole chunk: [P, CH, 2, NB]
                bxy = wpool.tile([P, CH * 2 * NB], f32, tag="bxy")
                bxy_v = bxy[:].rearrange("p (t d n) -> p t d n", t=CH, d=2)
                nc.vector.tensor_tensor(
                    out=bxy_v,
                    in0=attrf[:].rearrange("p (t d) -> p t d", d=2)
                        .unsqueeze(3).to_broadcast([P, CH, 2, NB]),
                    in1=cen[:].rearrange("p (d n) -> p d n", d=2)
                        .unsqueeze(1).to_broadcast([P, CH, 2, NB]),
                    op=AO.subtract)
                nc.scalar.activation(
                    out=bxy[:], in_=bxy[:], func=AF.Abs, scale=1.0 / DX)
                nc.scalar.activation(
                    out=bxy[:], in_=bxy[:], func=AF.Relu, bias=1.0, scale=-1.0)
                bmat = wpool.tile([P, CH * K], f32, tag="bmat")
                nc.vector.tensor_tensor(
                    out=bmat[:].rearrange("p (t a c) -> p t a c", t=CH, a=NB),
                    in0=bxy_v[:, :, 0, :].unsqueeze(3).to_broadcast([P, CH, NB, NB]),
                    in1=bxy_v[:, :, 1, :].unsqueeze(2).to_broadcast([P, CH, NB, NB]),
                    op=AO.mult)
                # segment one-hot S[p,t,q] = (seg[p,t] == q)
                smat = wpool.tile([P, CH * SEG], f32, tag="smat")
                nc.vector.tensor_tensor(
                    out=smat[:].rearrange("p (t q) -> p t q", t=CH),
                    in0=segf[:].unsqueeze(2).to_broadcast([P, CH, SEG]),
                    in1=io32[:].unsqueeze(1).to_broadcast([P, CH, SEG]),
                    op=AO.is_equal)

                stage16 = wpool.tile([P, NG * F], f16, tag="stage")
                for g in range(NG):
                    ft_ps = ftpool.tile([P, P], f32, tag="ft")
                    nc.tensor.transpose(
                        out=ft_ps[:], in_=feat[:, g * P:(g + 1) * P],
                        identity=ident[:])
                    ft_sb = wpool.tile([P, P], f32, tag="ftsb")
                    nc.scalar.activation(
                        out=ft_sb[:], in_=ft_ps[:], func=AF.Copy)
                    seg_ps = spool.tile([P, F], f32, tag="segps")
                    y_list, zz_list = [], []
                    for j in range(GRP):
                        y_ps = ypool.tile([P, K * F], f32, tag="y")
                        nc.tensor.matmul(
                            out=y_ps[:],
                            lhsT=ft_sb[32 * j:32 * (j + 1), :],
                            rhs=wf[32 * j:32 * (j + 1), :],
                            start=True, stop=True,
                            skip_group_check=True,
                            tile_position=(32 * j, 0))
                        y_list.append(y_ps)
                    for j in range(GRP):
                        tl = g * GRP + j
                        zz = zzpool.tile([P, K * F], f32, tag="zz")
                        nc.vector.tensor_tensor(
                            out=zz[:].rearrange("p (k o) -> p k o", k=K),
                            in0=y_list[j][:].rearrange("p (k o) -> p k o", k=K),
                            in1=bmat[:, tl * K:(tl + 1) * K]
                                .unsqueeze(2).to_broadcast([P, K, F]),
                            op=AO.mult)
                        zz_list.append(zz)
                    for j in range(GRP):
                        tl = g * GRP + j
                        for k in range(K):
                            nc.tensor.matmul(
                                out=seg_ps[32 * j:32 * (j + 1), :],
                                lhsT=smat[:, tl * SEG:(tl + 1) * SEG],
                                rhs=zz_list[j][:, k * F:(k + 1) * F],
                                start=(k == 0), stop=(k == K - 1),
                                skip_group_check=True,
                                tile_position=(0, 32 * j))
                    nc.scalar.activation(
                        out=stage16[:, g * F:(g + 1) * F], in_=seg_ps[:],
                        func=AF.Copy)
                    nc.gpsimd.indirect_dma_start(
                        out=out_d[:, :],
                        out_offset=bass.IndirectOffsetOnAxis(
                            ap=nid[:, g:g + 1], axis=0),
                        in_=stage16[:, g * F:(g + 1) * F], in_offset=None)
                if debug_dump and c == 0:
                    nc.sync.dma_start(dbg["idx"][:, :], idx[:])
                    nc.sync.dma_start(dbg["feat"][:, :], feat16[:])
                    nc.sync.dma_start(dbg["stage"][:, :], stage16[:])
                    nc.sync.dma_start(dbg["nid"][:, :], nid[:])

    nc.compile()
    return nc


def _executor(T, ROWS):
    key = (T, ROWS)
    if key in _EXEC:
        return _EXEC[key]
    import jax
    import jax.numpy as jnp
    from jax.sharding import Mesh, PartitionSpec
    from jax.experimental.shard_map import shard_map

    nc = _build_nc(T, ROWS)
    bass2jax.install_neuronx_cc_hook()
    partition_name = nc.partition_id_tensor.name if nc.partition_id_tensor else None
    in_names, out_names, out_avals = [], [], []
    for alloc in nc.m.functions[0].allocations:
        if not isinstance(alloc, mybir.MemoryLocationSet):
            continue
        name = alloc.memorylocations[0].name
        if alloc.kind == "ExternalInput":
            if name != partition_name:
                in_names.append(name)
        elif alloc.kind == "ExternalOutput":
            out_names.append(name)
            out_avals.append(jax.core.ShapedArray(
                tuple(alloc.tensor_shape), mybir.dt.np(alloc.dtype)))
    names_all = tuple(in_names + out_names +
                      ([partition_name] if partition_name else []))

    def _body(*args):
        operands = list(args)
        if partition_name is not None:
            operands.append(bass2jax.partition_id_tensor())
        return tuple(bass2jax._bass_exec_p.bind(
            *operands, out_avals=tuple(out_avals), in_names=names_all,
            out_names=tuple(out_names), lowering_input_output_aliases=(),
            sim_require_finite=True, sim_require_nnan=True, nc=nc))

    mesh = Mesh(np.asarray(jax.devices()[:NCORES]), ("core",))
    n_in = len(in_names)
    n_args = n_in + len(out_names)
    fn = jax.jit(
        shard_map(_body, mesh=mesh,
                  in_specs=(PartitionSpec("core"),) * n_args,
                  out_specs=(PartitionSpec("core"),) * len(out_names),
                  check_rep=False),
        donate_argnums=tuple(range(n_in, n_args)),
        keep_unused=True)
    # output operands must be donated (the NEFF's output region is the
    # operand buffer); refill them device-side (no host transfer) per call.
    from jax.sharding import NamedSharding
    shd = NamedSharding(mesh, PartitionSpec("core"))
    _SHD["shd"] = shd
    import functools

    @functools.partial(jax.jit, out_shardings=tuple([shd] * len(out_avals)))
    def zeros_maker():
        return tuple(jnp.zeros((NCORES * a.shape[0], *a.shape[1:]), a.dtype)
                     for a in out_avals)

    state = {"fn": fn, "in_names": in_names, "out_names": out_names,
             "out_avals": out_avals, "zeros_maker": zeros_maker,
             "zdev": list(zeros_maker())}
    _EXEC[key] = state
    return state


def _pack(dst, src, attr):
    """Sort edges by dst, split equally across cores, tile in 128-edge blocks.

    Returns the device input arrays plus (nrows, node_of_run) for unpacking.
    """
    E = dst.shape[0]
    Ec = -(-E // NCORES)
    T = -(-(-(-Ec // P)) // CH) * CH

    order = np.argsort(dst.astype(np.int32), kind="stable")
    dst_s = dst[order]
    src_s = src[order].astype(np.uint16)
    attr_s = attr[order].astype(np.float16)

    g = np.arange(E, dtype=np.int64)
    core = g // Ec
    pos = g - core * Ec
    tile_ = pos // P
    slot = pos - tile_ * P

    nb = np.empty(E, np.bool_)
    nb[0] = True
    np.not_equal(dst_s[1:], dst_s[:-1], out=nb[1:])
    nb[slot == 0] = True
    R = np.cumsum(nb) - 1                       # global run id of each edge
    run_start = np.flatnonzero(nb)
    node_of_run = dst_s[run_start]
    Rbase = R[np.minimum(np.arange(NCORES) * Ec, E - 1)]
    row = R - Rbase[core]                       # per-core output row
    nrows = np.empty(NCORES, np.int64)
    nrows[:-1] = Rbase[1:] - Rbase[:-1]
    nrows[-1] = R[-1] + 1 - Rbase[-1]

    tstart = core * Ec + tile_ * P              # global idx of tile start
    seg_e = R - R[tstart]
    max_seg = int(seg_e.max())
    assert max_seg < SEG, f"tile with {max_seg + 1} runs > {SEG}"

    src_il = np.zeros((NCORES, P, T), np.uint16)
    attr_il = np.full((NCORES, P, T, 2), DUMMY, np.float16)
    seg_il = np.full((NCORES, P, T), SEG - 1, np.uint8)
    src_il[core, slot, tile_] = src_s
    attr_il[core, slot, tile_] = attr_s
    seg_il[core, slot, tile_] = seg_e

    ROWS = ROWS_DEFAULT
    need = int(nrows.max()) + 1
    if need > ROWS:
        ROWS = -(-need // 512) * 512
    TRASH = ROWS - 1

    # per-(core, tile) first row + run count -> scatter row ids
    ts_idx = np.flatnonzero(slot == 0)          # first edge of each real tile
    te_idx = np.r_[ts_idx[1:] - 1, E - 1]       # last edge of each real tile
    R0 = np.zeros((NCORES, T), np.int64)
    cnt = np.zeros((NCORES, T), np.int64)
    R0[core[ts_idx], tile_[ts_idx]] = row[ts_idx]
    cnt[core[ts_idx], tile_[ts_idx]] = row[te_idx] - row[ts_idx] + 1
    qq = np.arange(SEG, dtype=np.int64)
    nid_mat = np.where(qq[None, None, :] < cnt[:, :, None],
                       R0[:, :, None] + qq[None, None, :], TRASH)
    nid_il = (nid_mat.astype(np.uint16)
              .reshape(NCORES, T // GRP, GRP, SEG)
              .transpose(0, 2, 3, 1)
              .reshape(NCORES, P, T // GRP))

    return {
        "T": T, "ROWS": ROWS,
        "src_il": src_il.reshape(NCORES * P, T),
        "attr_il": np.ascontiguousarray(attr_il.reshape(NCORES, P, T * 2))
                     .reshape(NCORES * P, T * 2),
        "seg_il": seg_il.reshape(NCORES * P, T),
        "nid_il": nid_il.reshape(NCORES * P, T // GRP),
        "nrows": nrows, "node_of_run": node_of_run,
    }


def kernel(x_i, x_j, edge_index, edge_attr, weight):
    global LAST_DISPATCH_NS
    import jax
    xj = np.asarray(x_j, np.float32)
    ei = np.asarray(edge_index)
    dst = np.ascontiguousarray(ei[0]).astype(np.int64)
    src = np.ascontiguousarray(ei[1]).astype(np.int64)
    attr = np.asarray(edge_attr, np.float32)
    w = np.asarray(weight, np.float32)

    # pack-independent inputs: start their (async) upload before packing so
    # the transfer overlaps the host-side index work
    xjs = np.ascontiguousarray(xj.astype(np.float16))
    assert xjs.shape == (NCORES * NSH, F)
    wflat = np.ascontiguousarray(
        w.transpose(2, 0, 1, 3).reshape(F, K * F).astype(np.float32))
    wf8 = np.tile(wflat, (NCORES, 1))
    shd = _SHD.get("shd")
    if shd is not None:
        xjs, wf8 = jax.device_put((xjs, wf8), (shd, shd))

    pk = _pack(dst, src, attr)
    T, ROWS = pk["T"], pk["ROWS"]

    st = _executor(T, ROWS)
    if shd is None:    # first call: executor now exists, upload for real
        shd = _SHD["shd"]
        xjs, wf8 = jax.device_put((xjs, wf8), (shd, shd))
    feed = {
        "xjs": xjs, "src_il": pk["src_il"], "attr_il": pk["attr_il"],
        "seg_il": pk["seg_il"], "nid_il": pk["nid_il"], "wflat": wf8,
    }
    args = [feed[n] for n in st["in_names"]] + st["zdev"]

    t0 = time.perf_counter()
    outs = st["fn"](*args)
    out_np = np.asarray(outs[0])
    LAST_DISPATCH_NS = int((time.perf_counter() - t0) * 1e9)
    st["zdev"] = list(st["zeros_maker"]())    # refill donated buffers

    acc = out_np.reshape(NCORES, ROWS, F).astype(np.float32)
    nrows = pk["nrows"]
    rows = np.concatenate([acc[c, :nrows[c]] for c in range(NCORES)], axis=0)
    out = np.zeros((N_NODES, F), np.float32)
    np.add.at(out, pk["node_of_run"], rows)
    return out


# revision 6
# speedup vs baseline: 1.7798x; 1.7798x over previous
"""BasisConv GNN message passing on 8 TRN2 NeuronCores.

Edge-parallel: edges sorted by dst, split equally (not node-aligned) across
8 cores; tiles are exactly 128 consecutive edges. Each (tile, run-of-equal-dst)
gets a unique output row per core; host merges rows (np.add.at) at the end.

Device per chunk (16 tiles): one batched indirect gather of x_j rows (f16,
from an AllGather'd replica), hat-basis + one-hot segment matrices on DVE,
PE pipeline (transpose -> feat@W -> *b -> one-hot segment-sum), one batched
indirect scatter of f16 rows to a compact per-core output.

Dispatch path: cached jax.jit(shard_map(bass_exec)) — compiled once per
(T, ROWS); per-call work is host->device upload of ~13MB of inputs, NEFF
execution, ~4MB readback.
"""

import sys
import time

for _p in ("/opt/trn_rl_repo", "/opt/pypackages"):
    if _p not in sys.path:
        sys.path.insert(0, _p)

import numpy as np

import concourse.bacc as bacc
import concourse.bass as bass
import concourse.mybir as mybir
import concourse.tile as tile
from concourse import bass2jax

N_NODES = 50000
F = 32          # feature dim (in == out)
NB = 4          # basis terms per dimension
K = NB * NB     # 16 mixture terms
P = 128         # edges per tile
SEG = 32        # max runs (output rows) per tile
CH = 16         # tiles per chunk (one gather/scatter DMA per chunk)
GRP = 4         # tiles per PE-transpose / PSUM column group
NG = CH // GRP
NCORES = 8
NSH = N_NODES // NCORES          # 6250 nodes per x_j shard
DX = 2.0 / (NB - 1)
DUMMY = 99.0                     # basis value is exactly 0 out there
ROWS_DEFAULT = 7168

f32 = mybir.dt.float32
f16 = mybir.dt.float16
i32 = mybir.dt.int32
u16 = mybir.dt.uint16
u8 = mybir.dt.uint8

LAST_DISPATCH_NS = None
_EXEC = {}
_SHD = {}


def _build_nc(T, ROWS, debug_dump=False):
    assert T % CH == 0
    nc = bacc.Bacc("TRN2", target_bir_lowering=False, debug=False,
                   enable_asserts=False, num_devices=NCORES)
    AO = mybir.AluOpType
    AF = mybir.ActivationFunctionType
    dbg = {}
    if debug_dump:
        dbg["xjf"] = nc.dram_tensor("dbg_xjf", [N_NODES, F], f16,
                                    kind="ExternalOutput")
        dbg["ident"] = nc.dram_tensor("dbg_ident", [P, P], f32,
                                      kind="ExternalOutput")
        dbg["cen"] = nc.dram_tensor("dbg_cen", [P, 2 * NB], f32,
                                    kind="ExternalOutput")
        dbg["io"] = nc.dram_tensor("dbg_io", [P, SEG], f32,
                                   kind="ExternalOutput")
        dbg["idx"] = nc.dram_tensor("dbg_idx", [P, CH], i32,
                                    kind="ExternalOutput")
        dbg["feat"] = nc.dram_tensor("dbg_feat", [P, CH * F], f16,
                                     kind="ExternalOutput")
        dbg["stage"] = nc.dram_tensor("dbg_stage", [P, NG * F], f16,
                                      kind="ExternalOutput")
        dbg["nid"] = nc.dram_tensor("dbg_nid", [P, NG], i32,
                                    kind="ExternalOutput")

    xjs_d = nc.dram_tensor("xjs", [NSH, F], f16, kind="ExternalInput")
    src_d = nc.dram_tensor("src_il", [P, T], u16, kind="ExternalInput")
    attr_d = nc.dram_tensor("attr_il", [P, T * 2], f16, kind="ExternalInput")
    seg_d = nc.dram_tensor("seg_il", [P, T], u8, kind="ExternalInput")
    nid_d = nc.dram_tensor("nid_il", [P, T // GRP], u16, kind="ExternalInput")
    wf_d = nc.dram_tensor("wflat", [F, K * F], f32, kind="ExternalInput")
    out_d = nc.dram_tensor("out", [ROWS, F], f16, kind="ExternalOutput")

    NCH = T // CH
    with tile.TileContext(nc) as tc:
        with (
            tc.tile_pool(name="dram", bufs=1, space="DRAM") as dpool,
            tc.tile_pool(name="const", bufs=1) as cpool,
            tc.tile_pool(name="io", bufs=2) as iopool,
            tc.tile_pool(name="work", bufs=2) as wpool,
            tc.tile_pool(name="zzp", bufs=6) as zzpool,
            tc.tile_pool(name="ftp", bufs=2, space="PSUM") as ftpool,
            tc.tile_pool(name="yp", bufs=4, space="PSUM") as ypool,
            tc.tile_pool(name="sp", bufs=2, space="PSUM") as spool,
        ):
            # x_j: shard -> replicated full table (f16) in local DRAM
            xj_bounce = dpool.tile([NSH, F], f16, tag="xjb")
            xj_full = dpool.tile([N_NODES, F], f16, tag="xjf")
            nc.gpsimd.dma_start(xj_bounce[:], xjs_d[:, :])
            nc.gpsimd.collective_compute(
                "AllGather", AO.bypass,
                replica_groups=[list(range(NCORES))],
                ins=[xj_bounce.opt()], outs=[xj_full.opt()])

            # constants: W replicated to 128 rows; ident/centers/iota on-device
            wf = cpool.tile([P, K * F], f32, tag="wf")
            for j in range(GRP):
                nc.sync.dma_start(wf[32 * j:32 * (j + 1), :], wf_d[:, :])

            ident_i = cpool.tile([P, P], i32, tag="identi")
            identf = cpool.tile([P, P], f32, tag="identf")
            ident = cpool.tile([P, P], f32, tag="ident")
            nc.gpsimd.iota(ident_i[:], [[1, P]], channel_multiplier=-1)
            nc.vector.tensor_copy(identf[:], ident_i[:])
            nc.vector.tensor_scalar(
                out=ident[:], in0=identf[:], scalar1=0.0, scalar2=None,
                op0=AO.is_equal)

            cen_i = cpool.tile([P, 2 * NB], i32, tag="ceni")
            cen = cpool.tile([P, 2 * NB], f32, tag="cen")
            nc.gpsimd.iota(cen_i[:], [[0, 2], [1, NB]], channel_multiplier=0)
            nc.vector.tensor_scalar(
                out=cen[:], in0=cen_i[:], scalar1=float(DX), scalar2=-1.0,
                op0=AO.mult, op1=AO.add)

            io_i = cpool.tile([P, SEG], i32, tag="ioi")
            io32 = cpool.tile([P, SEG], f32, tag="io")
            nc.gpsimd.iota(io_i[:], [[1, SEG]], channel_multiplier=0)
            nc.vector.tensor_copy(io32[:], io_i[:])

            if debug_dump:
                nc.sync.dma_start(dbg["xjf"][:, :], xj_full[:])
                nc.sync.dma_start(dbg["ident"][:, :], ident[:])
                nc.sync.dma_start(dbg["cen"][:, :], cen[:])
                nc.sync.dma_start(dbg["io"][:, :], io32[:])

            for c in range(NCH):
                ts = slice(c * CH, (c + 1) * CH)
                idx16 = iopool.tile([P, CH], u16, tag="idx16")
                attr16 = iopool.tile([P, CH * 2], f16, tag="attr16")
                seg8 = iopool.tile([P, CH], u8, tag="seg8")
                nid16 = iopool.tile([P, NG], u16, tag="nid16")
                nc.sync.dma_start(idx16[:], src_d[:, ts])
                nc.sync.dma_start(attr16[:], attr_d[:, c * CH * 2:(c + 1) * CH * 2])
                nc.sync.dma_start(seg8[:], seg_d[:, ts])
                nc.sync.dma_start(nid16[:], nid_d[:, c * NG:(c + 1) * NG])

                idx = iopool.tile([P, CH], i32, tag="idx")
                attrf = wpool.tile([P, CH * 2], f32, tag="attrf")
                segf = wpool.tile([P, CH], f32, tag="segf")
                nid = iopool.tile([P, NG], i32, tag="nid")
                nc.vector.tensor_copy(idx[:], idx16[:])
                nc.vector.tensor_copy(attrf[:], attr16[:])
                nc.vector.tensor_copy(segf[:], seg8[:])
                nc.vector.tensor_copy(nid[:], nid16[:])

                feat16 = wpool.tile([P, CH * F], f16, tag="feat16")
                for tl in range(CH):
                    nc.gpsimd.indirect_dma_start(
                        out=feat16[:, tl * F:(tl + 1) * F], out_offset=None,
                        in_=xj_full[:, :],
                        in_offset=bass.IndirectOffsetOnAxis(
                            ap=idx[:, tl:tl + 1], axis=0))
                feat = wpool.tile([P, CH * F], f32, tag="feat")
                nc.vector.tensor_copy(feat[:], feat16[:])

                # hat basis for the whole chunk: [P, CH, 2, NB]
                bxy = wpool.tile([P, CH * 2 * NB], f32, tag="bxy")
                bxy_v = bxy[:].rearrange("p (t d n) -> p t d n", t=CH, d=2)
                nc.vector.tensor_tensor(
                    out=bxy_v,
                    in0=attrf[:].rearrange("p (t d) -> p t d", d=2)
                        .unsqueeze(3).to_broadcast([P, CH, 2, NB]),
                    in1=cen[:].rearrange("p (d n) -> p d n", d=2)
                        .unsqueeze(1).to_broadcast([P, CH, 2, NB]),
                    op=AO.subtract)
                nc.scalar.activation(
                    out=bxy[:], in_=bxy[:], func=AF.Abs, scale=1.0 / DX)
                nc.scalar.activation(
                    out=bxy[:], in_=bxy[:], func=AF.Relu, bias=1.0, scale=-1.0)
                bmat = wpool.tile([P, CH * K], f32, tag="bmat")
                nc.vector.tensor_tensor(
                    out=bmat[:].rearrange("p (t a c) -> p t a c", t=CH, a=NB),
                    in0=bxy_v[:, :, 0, :].unsqueeze(3).to_broadcast([P, CH, NB, NB]),
                    in1=bxy_v[:, :, 1, :].unsqueeze(2).to_broadcast([P, CH, NB, NB]),
                    op=AO.mult)
                # segment one-hot S[p,t,q] = (seg[p,t] == q)
                smat = wpool.tile([P, CH * SEG], f32, tag="smat")
                nc.vector.tensor_tensor(
                    out=smat[:].rearrange("p (t q) -> p t q", t=CH),
                    in0=segf[:].unsqueeze(2).to_broadcast([P, CH, SEG]),
                    in1=io32[:].unsqueeze(1).to_broadcast([P, CH, SEG]),
                    op=AO.is_equal)

                stage16 = wpool.tile([P, NG * F], f16, tag="stage")
                for g in range(NG):
                    ft_ps = ftpool.tile([P, P], f32, tag="ft")
                    nc.tensor.transpose(
                        out=ft_ps[:], in_=feat[:, g * P:(g + 1) * P],
                        identity=ident[:])
                    ft_sb = wpool.tile([P, P], f32, tag="ftsb")
                    nc.scalar.activation(
                        out=ft_sb[:], in_=ft_ps[:], func=AF.Copy)
                    seg_ps = spool.tile([P, F], f32, tag="segps")
                    y_list, zz_list = [], []
                    for j in range(GRP):
                        y_ps = ypool.tile([P, K * F], f32, tag="y")
                        nc.tensor.matmul(
                            out=y_ps[:],
                            lhsT=ft_sb[32 * j:32 * (j + 1), :],
                            rhs=wf[32 * j:32 * (j + 1), :],
                            start=True, stop=True,
                            skip_group_check=True,
                            tile_position=(32 * j, 0))
                        y_list.append(y_ps)
                    for j in range(GRP):
                        tl = g * GRP + j
                        zz = zzpool.tile([P, K * F], f32, tag="zz")
                        nc.vector.tensor_tensor(
                            out=zz[:].rearrange("p (k o) -> p k o", k=K),
                            in0=y_list[j][:].rearrange("p (k o) -> p k o", k=K),
                            in1=bmat[:, tl * K:(tl + 1) * K]
                                .unsqueeze(2).to_broadcast([P, K, F]),
                            op=AO.mult)
                        zz_list.append(zz)
                    for j in range(GRP):
                        tl = g * GRP + j
                        for k in range(K):
                            nc.tensor.matmul(
                                out=seg_ps[32 * j:32 * (j + 1), :],
                                lhsT=smat[:, tl * SEG:(tl + 1) * SEG],
                                rhs=zz_list[j][:, k * F:(k + 1) * F],
                                start=(k == 0), stop=(k == K - 1),
                                skip_group_check=True,
                                tile_position=(0, 32 * j))
                    nc.scalar.activation(
                        out=stage16[:, g * F:(g + 1) * F], in_=seg_ps[:],
                        func=AF.Copy)
                    nc.gpsimd.indirect_dma_start(
                        out=out_d[:, :],
                        out_offset=bass.IndirectOffsetOnAxis(
                            ap=nid[:, g:g + 1], axis=0),
                        in_=stage16[:, g * F:(g + 1) * F], in_offset=None)
                if debug_dump and c == 0:
                    nc.sync.dma_start(dbg["idx"][:, :], idx[:])
                    nc.sync.dma_start(dbg["feat"][:, :], feat16[:])
                    nc.sync.dma_start(dbg["stage"][:, :], stage16[:])
                    nc.sync.dma_start(dbg["nid"][:, :], nid[:])

    nc.compile()
    return nc


def _executor(T, ROWS):
    key = (T, ROWS)
    if key in _EXEC:
        return _EXEC[key]
    import jax
    import jax.numpy as jnp
    from jax.sharding import Mesh, PartitionSpec
    from jax.experimental.shard_map import shard_map

    nc = _build_nc(T, ROWS)
    bass2jax.install_neuronx_cc_hook()
    partition_name = nc.partition_id_tensor.name if nc.partition_id_tensor else None
    in_names, out_names, out_avals = [], [], []
    for alloc in nc.m.functions[0].allocations:
        if not isinstance(alloc, mybir.MemoryLocationSet):
            continue
        name = alloc.memorylocations[0].name
        if alloc.kind == "ExternalInput":
            if name != partition_name:
                in_names.append(name)
        elif alloc.kind == "ExternalOutput":
            out_names.append(name)
            out_avals.append(jax.core.ShapedArray(
                tuple(alloc.tensor_shape), mybir.dt.np(alloc.dtype)))
    names_all = tuple(in_names + out_names +
                      ([partition_name] if partition_name else []))

    def _body(*args):
        operands = list(args)
        if partition_name is not None:
            operands.append(bass2jax.partition_id_tensor())
        return tuple(bass2jax._bass_exec_p.bind(
            *operands, out_avals=tuple(out_avals), in_names=names_all,
            out_names=tuple(out_names), lowering_input_output_aliases=(),
            sim_require_finite=True, sim_require_nnan=True, nc=nc))

    mesh = Mesh(np.asarray(jax.devices()[:NCORES]), ("core",))
    n_in = len(in_names)
    n_args = n_in + len(out_names)
    fn = jax.jit(
        shard_map(_body, mesh=mesh,
                  in_specs=(PartitionSpec("core"),) * n_args,
                  out_specs=(PartitionSpec("core"),) * len(out_names),
                  check_rep=False),
        donate_argnums=tuple(range(n_in, n_args)),
        keep_unused=True)
    # output operands must be donated (the NEFF's output region is the
    # operand buffer); refill them device-side (no host transfer) per call.
    from jax.sharding import NamedSharding
    shd = NamedSharding(mesh, PartitionSpec("core"))
    _SHD["shd"] = shd
    import functools

    @functools.partial(jax.jit, out_shardings=tuple([shd] * len(out_avals)))
    def zeros_maker():
        return tuple(jnp.zeros((NCORES * a.shape[0], *a.shape[1:]), a.dtype)
                     for a in out_avals)

    state = {"fn": fn, "in_names": in_names, "out_names": out_names,
             "out_avals": out_avals, "zeros_maker": zeros_maker,
             "zdev": list(zeros_maker())}
    _EXEC[key] = state
    return state


def _pack_stage1(dst, src, attr):
    """Sort edges by dst, split equally across cores, tile in 128-edge blocks.

    Produces src_il/attr_il (upload can start while stage2 runs).
    """
    E = dst.shape[0]
    Ec = -(-E // NCORES)
    T = -(-(-(-Ec // P)) // CH) * CH

    order = np.argsort(dst.astype(np.int32), kind="stable")
    dst_s = dst[order]
    src_s = src[order].astype(np.uint16)
    attr_s = attr[order].astype(np.float16)

    g = np.arange(E, dtype=np.int64)
    core = g // Ec
    pos = g - core * Ec
    tile_ = pos // P
    slot = pos - tile_ * P

    src_il = np.zeros((NCORES, P, T), np.uint16)
    attr_il = np.full((NCORES, P, T, 2), DUMMY, np.float16)
    src_il[core, slot, tile_] = src_s
    attr_il[core, slot, tile_] = attr_s

    return {
        "E": E, "Ec": Ec, "T": T, "dst_s": dst_s,
        "core": core, "tile_": tile_, "slot": slot,
        "src_il": src_il.reshape(NCORES * P, T),
        "attr_il": np.ascontiguousarray(attr_il.reshape(NCORES, P, T * 2))
                     .reshape(NCORES * P, T * 2),
    }


def _pack_stage2(s1):
    """Run/segment structure: seg_il, nid_il and the unpack metadata."""
    E, Ec, T = s1["E"], s1["Ec"], s1["T"]
    dst_s = s1["dst_s"]
    core, tile_, slot = s1["core"], s1["tile_"], s1["slot"]

    nb = np.empty(E, np.bool_)
    nb[0] = True
    np.not_equal(dst_s[1:], dst_s[:-1], out=nb[1:])
    nb[slot == 0] = True
    R = np.cumsum(nb) - 1                       # global run id of each edge
    run_start = np.flatnonzero(nb)
    node_of_run = dst_s[run_start]
    Rbase = R[np.minimum(np.arange(NCORES) * Ec, E - 1)]
    row = R - Rbase[core]                       # per-core output row
    nrows = np.empty(NCORES, np.int64)
    nrows[:-1] = Rbase[1:] - Rbase[:-1]
    nrows[-1] = R[-1] + 1 - Rbase[-1]

    tstart = core * Ec + tile_ * P              # global idx of tile start
    seg_e = R - R[tstart]
    max_seg = int(seg_e.max())
    assert max_seg < SEG, f"tile with {max_seg + 1} runs > {SEG}"

    seg_il = np.full((NCORES, P, T), SEG - 1, np.uint8)
    seg_il[core, slot, tile_] = seg_e

    ROWS = ROWS_DEFAULT
    need = int(nrows.max()) + 1
    if need > ROWS:
        ROWS = -(-need // 512) * 512
    TRASH = ROWS - 1

    # per-(core, tile) first row + run count -> scatter row ids
    ts_idx = np.flatnonzero(slot == 0)          # first edge of each real tile
    te_idx = np.r_[ts_idx[1:] - 1, E - 1]       # last edge of each real tile
    R0 = np.zeros((NCORES, T), np.int64)
    cnt = np.zeros((NCORES, T), np.int64)
    R0[core[ts_idx], tile_[ts_idx]] = row[ts_idx]
    cnt[core[ts_idx], tile_[ts_idx]] = row[te_idx] - row[ts_idx] + 1
    qq = np.arange(SEG, dtype=np.int64)
    nid_mat = np.where(qq[None, None, :] < cnt[:, :, None],
                       R0[:, :, None] + qq[None, None, :], TRASH)
    nid_il = (nid_mat.astype(np.uint16)
              .reshape(NCORES, T // GRP, GRP, SEG)
              .transpose(0, 2, 3, 1)
              .reshape(NCORES, P, T // GRP))

    return {
        "T": T, "ROWS": ROWS,
        "seg_il": seg_il.reshape(NCORES * P, T),
        "nid_il": nid_il.reshape(NCORES * P, T // GRP),
        "nrows": nrows, "node_of_run": node_of_run,
    }


def _pack(dst, src, attr):
    s1 = _pack_stage1(dst, src, attr)
    s2 = _pack_stage2(s1)
    return {**s1, **s2}


def kernel(x_i, x_j, edge_index, edge_attr, weight):
    global LAST_DISPATCH_NS
    import jax
    xj = np.asarray(x_j, np.float32)
    ei = np.asarray(edge_index)
    dst = np.ascontiguousarray(ei[0]).astype(np.int64)
    src = np.ascontiguousarray(ei[1]).astype(np.int64)
    attr = np.asarray(edge_attr, np.float32)
    w = np.asarray(weight, np.float32)

    # pack-independent inputs: start their (async) upload before packing so
    # the transfer overlaps the host-side index work
    xjs = np.ascontiguousarray(xj.astype(np.float16))
    assert xjs.shape == (NCORES * NSH, F)
    wflat = np.ascontiguousarray(
        w.transpose(2, 0, 1, 3).reshape(F, K * F).astype(np.float32))
    wf8 = np.tile(wflat, (NCORES, 1))
    shd = _SHD.get("shd")
    if shd is not None:
        xjs, wf8 = jax.device_put((xjs, wf8), (shd, shd))

    s1 = _pack_stage1(dst, src, attr)
    src_a, attr_a = s1["src_il"], s1["attr_il"]
    if shd is not None:
        # async: src/attr upload overlaps the stage-2 index computation
        src_a, attr_a = jax.device_put((src_a, attr_a), (shd, shd))
    pk = _pack_stage2(s1)
    T, ROWS = s1["T"], pk["ROWS"]

    st = _executor(T, ROWS)
    seg_a, nid_a = pk["seg_il"], pk["nid_il"]
    if shd is None:    # first call: executor now exists, upload for real
        shd = _SHD["shd"]
        xjs, wf8 = jax.device_put((xjs, wf8), (shd, shd))
        src_a, attr_a = jax.device_put((src_a, attr_a), (shd, shd))
    seg_a, nid_a = jax.device_put((seg_a, nid_a), (shd, shd))
    feed = {
        "xjs": xjs, "src_il": src_a, "attr_il": attr_a,
        "seg_il": seg_a, "nid_il": nid_a, "wflat": wf8,
    }
    args = [feed[n] for n in st["in_names"]] + st["zdev"]

    t0 = time.perf_counter()
    outs = st["fn"](*args)
    out_np = np.asarray(outs[0])
    LAST_DISPATCH_NS = int((time.perf_counter() - t0) * 1e9)
    st["zdev"] = list(st["zeros_maker"]())    # refill donated buffers

    acc = out_np.reshape(NCORES, ROWS, F).astype(np.float32)
    nrows = pk["nrows"]
    rows = np.concatenate([acc[c, :nrows[c]] for c in range(NCORES)], axis=0)
    out = np.zeros((N_NODES, F), np.float32)
    np.add.at(out, pk["node_of_run"], rows)
    return out
